# revision 14
# baseline (speedup 1.0000x reference)
"""BiUTE kernel for Trainium2, 8-core data-parallel over batch.

Math (per batch element b, T=128, N=12, D=1024, F=2D=2048):
  u = Wq.sum(0)                                  [D]
  w[t,n]  = sum_d feat[t,n,d] * u[d]             [T,N]
  g[t,d]  = sum_n w[t,n] * feat[t,n,d]           [T,D]
  f[t,d]  = max_n feat[t,n,d]                    [T,D]
  n = [g | f]                                    [T,F]
  tb = n @ Wtb.T ; pb = n @ Wpb.T ; gb = n @ Wgb.T
  sb = (tb @ pb.T) * scale ; out_b = (sb*lower) @ gb
  (same for 'after' branch with upper mask)
  out = n + out_b + out_a                        [T,F]

Schedule (per core, NB=2 batches): features stream in n-chunks and
phase A runs chunk-by-chunk on DVE in fp16 (f-max for batch 0 on
GpSimd); weight quarters are pre-packed host-side to SBUF layout and
stream continuously through a 5-deep pool in consumption order
(wtp_b, wg_b, wg_a, wtp_a); pass1_b starts as soon as n is
transposed, so the tensor engine runs dense from ~20us on. Final sums
accumulate in-place into the fp16 n buffer which is DMA'd out and
cast to fp32 on host.
"""

import numpy as np

import concourse.mybir as mybir
import concourse.tile as tile
from concourse import bacc
from concourse.bass_utils import run_bass_kernel_spmd

F32 = mybir.dt.float32
F16 = mybir.dt.float16

B, T, NP, D = 16, 128, 12, 1024
F = 2 * D                      # 2048
FC_ORDER = list(range(16))     # g-half chunks first (phase A emits g first)
NB = 2                         # batch elements per core
NCORES = 8
NFC = F // 128                 # 16 f-chunks of nT
SCALE = 1.0 / float(np.sqrt(F))

_CACHE = {}
_PROFILE = {"trace": False, "result": None}


def _build():
    nc = bacc.Bacc("TRN2", target_bir_lowering=False, debug=False)
    mult = mybir.AluOpType.mult
    add = mybir.AluOpType.add

    featd = nc.dram_tensor("feat", [NB, NP, T, D], F16, kind="ExternalInput")
    ud = nc.dram_tensor("u", [1, D], F16, kind="ExternalInput")
    mbd = nc.dram_tensor("maskb", [T, T], F32, kind="ExternalInput")
    mad = nc.dram_tensor("maska", [T, T], F32, kind="ExternalInput")
    identd = nc.dram_tensor("ident", [128, 128], F16, kind="ExternalInput")
    # weights pre-packed host-side: [quarter, 128, NFC, 512]
    wtpbd = nc.dram_tensor("wtp_b", [4, 128, NFC, 512], F16,
                           kind="ExternalInput")
    wgbd = nc.dram_tensor("wg_b", [4, 128, NFC, 512], F16,
                          kind="ExternalInput")
    wtpad = nc.dram_tensor("wtp_a", [4, 128, NFC, 512], F16,
                           kind="ExternalInput")
    wgad = nc.dram_tensor("wg_a", [4, 128, NFC, 512], F16,
                          kind="ExternalInput")
    outd = nc.dram_tensor("out", [NB, T, F], F32, kind="ExternalOutput")

    with tile.TileContext(nc) as tc:
        with (
            tc.tile_pool(name="consts", bufs=1) as consts,
            tc.tile_pool(name="npool", bufs=1) as npool,
            tc.tile_pool(name="ntpool", bufs=1) as ntpool,
            tc.tile_pool(name="wres", bufs=5) as wsp,
            tc.tile_pool(name="drains", bufs=1) as drp,
            tc.tile_pool(name="sbp", bufs=2) as sbp,
            tc.tile_pool(name="featp", bufs=12) as featp,
            tc.tile_pool(name="tmpp", bufs=3) as tmpp,
            tc.tile_pool(name="outp", bufs=1) as outp,
            tc.tile_pool(name="aw", bufs=1) as awp,
            tc.tile_pool(name="ptp", bufs=2, space="PSUM") as ptp,
            tc.tile_pool(name="psbp", bufs=2, space="PSUM") as psbp,
            tc.tile_pool(name="bigp", bufs=4, space="PSUM") as bigp,
        ):
            u_sb = consts.tile([128, D], F16)
            nc.gpsimd.dma_start(out=u_sb[:], in_=ud[:].to_broadcast((128, D)))
            mb_sb = consts.tile([T, T], F32)
            nc.gpsimd.dma_start(out=mb_sb[:], in_=mbd[:])
            ma_sb = consts.tile([T, T], F32)
            nc.gpsimd.dma_start(out=ma_sb[:], in_=mad[:])
            ident = consts.tile([128, 128], F16)
            nc.gpsimd.dma_start(out=ident[:], in_=identd[:])

            n_sb = [
                npool.tile([T, F], F16, tag=f"n{b}", name=f"n{b}")
                for b in range(NB)
            ]
            nT = ntpool.tile([128, NFC, NB * T], F16)

            # Feature chunk loads: batch 0 split across both HWDGE rings
            # first, then batch 1; weight loads queue behind on both rings.
            feats = []  # feats[b][c] -> tile
            for b in range(NB):
                fcs = []
                for c in range(NP):
                    ft = featp.tile([T, D], F16, tag="feat",
                                    name=f"feat{b}_{c}")
                    eng = nc.sync if c % 2 == 0 else nc.scalar
                    eng.dma_start(out=ft[:], in_=featd[b, c])
                    fcs.append(ft)
                feats.append(fcs)

            # All 16 weight-quarter loads, in consumption order, 5-deep pool.
            def load_w_quarter(src, qc, name, eng):
                wh = wsp.tile([128, NFC, 512], F16, tag="w", name=name)
                eng.dma_start(out=wh[:], in_=src[qc])
                return wh

            w_handles = []
            qi = 0
            for src, sfx in ((wgbd, "g_b"), (wgad, "g_a"),
                             (wtpbd, "tp_b"), (wtpad, "tp_a")):
                for qc in range(4):
                    eng = nc.sync if qi % 2 == 0 else nc.scalar
                    w_handles.append(
                        load_w_quarter(src, qc, f"w{sfx}_{qc}", eng))
                    qi += 1
            wg_b_h = w_handles[0:4]
            wg_a_h = w_handles[4:8]
            wtp_b_h = w_handles[8:12]
            wtp_a_h = w_handles[12:16]

            # ---------------- Phase A ----------------
            wvecs = []
            scr = awp.tile([T, D], F16, tag="scr", name="scr")

            def emit_phase_a(b):
                """DVE: w-STT chain, then per-chunk g-adds and f-maxes;
                GpSimd: the per-chunk scale tmp_c = feat_c * w_c."""
                wvec = awp.tile([T, NP], F32, tag=f"wvec{b}",
                                name=f"wvec{b}")
                g_ap = n_sb[b][:, :D]
                f_ap = n_sb[b][:, D:]
                for c in range(NP):
                    nc.vector.scalar_tensor_tensor(
                        out=scr[:], in0=feats[b][c][:], scalar=1.0,
                        in1=u_sb[:], op0=mult, op1=mult,
                        accum_out=wvec[:, c : c + 1],
                    )
                # GpSimd scales chunks as their w lands; c=0 inits g.
                tmps = [None] * NP
                nc.gpsimd.tensor_scalar(
                    out=g_ap, in0=feats[b][0][:], scalar1=wvec[:, 0:1],
                    scalar2=None, op0=mult,
                )
                for c in range(1, NP):
                    t_ = tmpp.tile([T, D], F16, tag="tmp", name=f"tmp{b}_{c}")
                    nc.gpsimd.tensor_scalar(
                        out=t_[:], in0=feats[b][c][:],
                        scalar1=wvec[:, c : c + 1], scalar2=None, op0=mult,
                    )
                    tmps[c] = t_
                # DVE: g accumulation + f maxes, interleaved per chunk.
                nc.vector.tensor_max(f_ap, feats[b][0][:], feats[b][1][:])
                for c in range(1, NP):
                    nc.vector.tensor_add(g_ap, g_ap, tmps[c][:])
                    if c >= 2:
                        nc.vector.tensor_max(f_ap, f_ap, feats[b][c][:])
                wvecs.append(wvec)

            def emit_transp(b, fc):
                pt = ptp.tile([128, 128], F16, tag="pt", name="pt")
                nc.tensor.transpose(
                    pt[:], n_sb[b][:, 128 * fc : 128 * (fc + 1)], ident[:]
                )
                nc.scalar.copy(nT[:, fc, T * b : T * (b + 1)], pt[:])

            for b in range(NB):
                emit_phase_a(b)
                for fc in range(NFC):
                    emit_transp(b, fc)

            # ---------------- Matmul passes ----------------
            def emit_pass1(whs, tp2):
                """tp2[:, 4qc+e4, :] = (Wt|Wp quarter).T-proj, e-major."""
                for qc in range(4):
                    for e4 in range(4):
                        p1 = bigp.tile([128, NB * T], F32, tag="big",
                                       name=f"p1_{qc}_{e4}")
                        for i, fc in enumerate(FC_ORDER):
                            nc.tensor.matmul(
                                p1[:],
                                whs[qc][:, fc, 128 * e4 : 128 * (e4 + 1)],
                                nT[:, fc, :],
                                start=(i == 0),
                                stop=(i == NFC - 1),
                            )
                        nc.scalar.copy(tp2[:, 4 * qc + e4, :], p1[:])

            def emit_pass2(whs, b, gb16):
                """gb16[b] = n_b @ Wg.T  (t-major)."""
                for qc in range(4):
                    psg = bigp.tile([128, 512], F32, tag="big",
                                    name=f"psg{b}_{qc}")
                    for i, fc in enumerate(FC_ORDER):
                        nc.tensor.matmul(
                            psg[:],
                            nT[:, fc, T * b : T * (b + 1)],
                            whs[qc][:, fc, :],
                            start=(i == 0),
                            stop=(i == NFC - 1),
                        )
                    nc.scalar.copy(
                        gb16[b][:, 512 * qc : 512 * (qc + 1)], psg[:]
                    )

            def emit_pass3(tp2, gb16, mask_sb, first):
                for b in range(NB):
                    psb = psbp.tile([T, T], F32, tag="psb", name="psb")
                    for ec in range(8):
                        nc.tensor.matmul(
                            psb[:],
                            tp2[:, 8 + ec, T * b : T * (b + 1)],
                            tp2[:, ec, T * b : T * (b + 1)],
                            start=(ec == 0),
                            stop=(ec == 7),
                        )
                    sbm = sbp.tile([T, T], F16, tag="sbm", name="sbm")
                    nc.vector.scalar_tensor_tensor(
                        out=sbm[:], in0=psb[:], scalar=1.0, in1=mask_sb[:],
                        op0=mult, op1=mult,
                    )
                    for h4 in range(4):
                        po = bigp.tile([T, 512], F32, tag="big",
                                       name=f"po{b}_{h4}")
                        nc.tensor.matmul(
                            po[:],
                            sbm[:],
                            gb16[b][:, 512 * h4 : 512 * (h4 + 1)],
                            start=True,
                            stop=True,
                        )
                        sl = slice(512 * h4, 512 * (h4 + 1))
                        if first:
                            nc.vector.tensor_add(
                                out32[b][:, sl], n_sb[b][:, sl], po[:]
                            )
                        else:
                            nc.vector.tensor_add(
                                out32[b][:, sl], out32[b][:, sl], po[:]
                            )

            tp2_b = drp.tile([128, 16, NB * T], F16, tag="tp2b", name="tp2b")
            gb16_b = [drp.tile([T, F], F16, tag=f"gb{b}b", name=f"gb{b}b")
                      for b in range(NB)]
            tp2_a = drp.tile([128, 16, NB * T], F16, tag="tp2a", name="tp2a")
            gb16_a = [drp.tile([T, F], F16, tag=f"gb{b}a", name=f"gb{b}a")
                      for b in range(NB)]
            out32 = [outp.tile([T, F], F32, tag=f"out{b}", name=f"out{b}")
                     for b in range(NB)]

            emit_pass2(wg_b_h, 0, gb16_b)
            emit_pass2(wg_b_h, 1, gb16_b)
            emit_pass2(wg_a_h, 0, gb16_a)
            emit_pass2(wg_a_h, 1, gb16_a)
            emit_pass1(wtp_b_h, tp2_b)
            emit_pass3(tp2_b, gb16_b, mb_sb, first=True)
            emit_pass1(wtp_a_h, tp2_a)
            emit_pass3(tp2_a, gb16_a, ma_sb, first=False)

            for b in range(NB):
                nc.sync.dma_start(out=outd[b], in_=out32[b][:])

    nc.compile()
    return nc


def _host_prep(features, Wq, Wtb, Wpb, Wgb, Wta, Wpa, Wga):
    f32 = np.float32
    f16 = np.float16
    # [B, T, NP, D] -> [B, NP, T, D] n-major for chunked phase A
    feat = np.ascontiguousarray(
        np.asarray(features, f32).transpose(0, 2, 1, 3).astype(f16)
    )
    u = np.asarray(Wq, f32).sum(axis=0)[None, :]

    def wt(w):  # [e, f] -> [f, e] fp16 contiguous
        return np.ascontiguousarray(np.asarray(w, f32).T.astype(f16))

    def pack(w):  # [F, E] -> [E//512, 128, NFC, 512] SBUF-layout quarters
        q = w.reshape(NFC, 128, -1)  # [c, p, E]
        ne = q.shape[2] // 512
        # quarter qc: rows (c p), cols 512*qc:512*(qc+1) -> [p, c, 512]
        out = np.empty((ne, 128, NFC, 512), f16)
        for qc in range(ne):
            out[qc] = q[:, :, 512 * qc : 512 * (qc + 1)].transpose(1, 0, 2)
        return np.ascontiguousarray(out)

    wtp_b = pack(np.concatenate([wt(Wtb), wt(Wpb)], axis=1))
    wtp_a = pack(np.concatenate([wt(Wta), wt(Wpa)], axis=1))
    wg_b = pack(wt(Wgb))
    wg_a = pack(wt(Wga))

    idx = np.arange(T)
    # masks indexed [j, i] (psb holds sb transposed)
    maskb = (SCALE * (idx[None, :] > idx[:, None])).astype(f32)
    maska = (SCALE * (idx[None, :] < idx[:, None])).astype(f32)
    ident = np.eye(128, dtype=f16)

    shared = {
        "u": u.astype(f16),
        "maskb": maskb,
        "maska": maska,
        "ident": ident,
        "wtp_b": wtp_b,
        "wg_b": wg_b,
        "wtp_a": wtp_a,
        "wg_a": wg_a,
    }
    feat16 = feat.reshape(NCORES, NB, NP, T, D)
    return shared, feat16


def kernel(**inputs) -> np.ndarray:
    if "nc" not in _CACHE:
        _CACHE["nc"] = _build()
    nc = _CACHE["nc"]

    shared, feat16 = _host_prep(**inputs)
    in_maps = [dict(shared, feat=feat16[c]) for c in range(NCORES)]
    res = run_bass_kernel_spmd(
        nc, in_maps, core_ids=list(range(NCORES)), trace=_PROFILE["trace"]
    )
    _PROFILE["result"] = res
    out = np.stack([res.results[c]["out"] for c in range(NCORES)], axis=0)
    return out.reshape(B, T, F).astype(np.float32)


# revision 16
# speedup vs baseline: 2.4661x; 2.4661x over previous
"""BiUTE kernel for Trainium2, 8-core data-parallel over batch.

Math (per batch element b, T=128, N=12, D=1024, F=2D=2048):
  u = Wq.sum(0)                                  [D]
  w[t,n]  = sum_d feat[t,n,d] * u[d]             [T,N]
  g[t,d]  = sum_n w[t,n] * feat[t,n,d]           [T,D]
  f[t,d]  = max_n feat[t,n,d]                    [T,D]
  n = [g | f]                                    [T,F]
  tb = n @ Wtb.T ; pb = n @ Wpb.T ; gb = n @ Wgb.T
  sb = (tb @ pb.T) * scale ; out_b = (sb*lower) @ gb
  (same for 'after' branch with upper mask)
  out = n + out_b + out_a                        [T,F]

Schedule (per core, NB=2 batches): features stream in n-chunks and
phase A runs chunk-by-chunk on DVE in fp16 (f-max for batch 0 on
GpSimd); weight quarters are pre-packed host-side to SBUF layout and
stream continuously through a 5-deep pool in consumption order
(wtp_b, wg_b, wg_a, wtp_a); pass1_b starts as soon as n is
transposed, so the tensor engine runs dense from ~20us on. Final sums
accumulate in-place into the fp16 n buffer which is DMA'd out and
cast to fp32 on host.
"""

import numpy as np

import concourse.mybir as mybir
import concourse.tile as tile
from concourse import bacc
from concourse.bass_utils import run_bass_kernel_spmd

F32 = mybir.dt.float32
F16 = mybir.dt.float16

B, T, NP, D = 16, 128, 12, 1024
F = 2 * D                      # 2048
FC_ORDER = list(range(16))     # g-half chunks first (phase A emits g first)
NB = 2                         # batch elements per core
NCORES = 8
NFC = F // 128                 # 16 f-chunks of nT
SCALE = 1.0 / float(np.sqrt(F))

_CACHE = {}
_PROFILE = {"trace": False, "result": None}


def _build():
    nc = bacc.Bacc("TRN2", target_bir_lowering=False, debug=False)
    mult = mybir.AluOpType.mult
    add = mybir.AluOpType.add

    featd = nc.dram_tensor("feat", [NB, NP, T, D], F16, kind="ExternalInput")
    ud = nc.dram_tensor("u", [1, D], F16, kind="ExternalInput")
    mbd = nc.dram_tensor("maskb", [T, T], F32, kind="ExternalInput")
    mad = nc.dram_tensor("maska", [T, T], F32, kind="ExternalInput")
    identd = nc.dram_tensor("ident", [128, 128], F16, kind="ExternalInput")
    # weights pre-packed host-side: [quarter, 128, NFC, 512]
    wtpbd = nc.dram_tensor("wtp_b", [4, 128, NFC, 512], F16,
                           kind="ExternalInput")
    wgbd = nc.dram_tensor("wg_b", [4, 128, NFC, 512], F16,
                          kind="ExternalInput")
    wtpad = nc.dram_tensor("wtp_a", [4, 128, NFC, 512], F16,
                           kind="ExternalInput")
    wgad = nc.dram_tensor("wg_a", [4, 128, NFC, 512], F16,
                          kind="ExternalInput")
    outd = nc.dram_tensor("out", [NB, T, F], F32, kind="ExternalOutput")

    with tile.TileContext(nc) as tc:
        with (
            tc.tile_pool(name="consts", bufs=1) as consts,
            tc.tile_pool(name="npool", bufs=1) as npool,
            tc.tile_pool(name="ntpool", bufs=1) as ntpool,
            tc.tile_pool(name="wres", bufs=5) as wsp,
            tc.tile_pool(name="drains", bufs=1) as drp,
            tc.tile_pool(name="sbp", bufs=2) as sbp,
            tc.tile_pool(name="featp", bufs=12) as featp,
            tc.tile_pool(name="outp", bufs=1) as outp,
            tc.tile_pool(name="aw", bufs=1) as awp,
            tc.tile_pool(name="ptp", bufs=2, space="PSUM") as ptp,
            tc.tile_pool(name="psbp", bufs=2, space="PSUM") as psbp,
            tc.tile_pool(name="bigp", bufs=4, space="PSUM") as bigp,
        ):
            u_sb = consts.tile([128, D], F16)
            nc.gpsimd.dma_start(out=u_sb[:], in_=ud[:].to_broadcast((128, D)))
            mb_sb = consts.tile([T, T], F32)
            nc.gpsimd.dma_start(out=mb_sb[:], in_=mbd[:])
            ma_sb = consts.tile([T, T], F32)
            nc.gpsimd.dma_start(out=ma_sb[:], in_=mad[:])
            ident = consts.tile([128, 128], F16)
            nc.gpsimd.dma_start(out=ident[:], in_=identd[:])

            n_sb = [
                npool.tile([T, F], F16, tag=f"n{b}", name=f"n{b}")
                for b in range(NB)
            ]
            nT = ntpool.tile([128, NFC, NB * T], F16)

            # Feature chunk loads: batch 0 split across both HWDGE rings
            # first, then batch 1; weight loads queue behind on both rings.
            feats = []  # feats[b][c] -> tile
            for b in range(NB):
                fcs = []
                for c in range(NP):
                    ft = featp.tile([T, D], F16, tag="feat",
                                    name=f"feat{b}_{c}")
                    eng = nc.sync if c % 2 == 0 else nc.scalar
                    eng.dma_start(out=ft[:], in_=featd[b, c])
                    fcs.append(ft)
                feats.append(fcs)

            # All 16 weight-quarter loads, in consumption order, 5-deep pool.
            def load_w_quarter(src, qc, name, eng):
                wh = wsp.tile([128, NFC, 512], F16, tag="w", name=name)
                eng.dma_start(out=wh[:], in_=src[qc])
                return wh

            w_handles = []
            qi = 0
            for src, sfx in ((wgbd, "g_b"), (wgad, "g_a"),
                             (wtpbd, "tp_b"), (wtpad, "tp_a")):
                for qc in range(4):
                    eng = nc.sync if qi % 2 == 0 else nc.scalar
                    w_handles.append(
                        load_w_quarter(src, qc, f"w{sfx}_{qc}", eng))
                    qi += 1
            wg_b_h = w_handles[0:4]
            wg_a_h = w_handles[4:8]
            wtp_b_h = w_handles[8:12]
            wtp_a_h = w_handles[12:16]

            # ---------------- Phase A ----------------
            wvecs = []
            scr = awp.tile([T, D], F16, tag="scr", name="scr")

            def emit_transp(b, fc):
                pt = ptp.tile([128, 128], F16, tag="pt", name="pt")
                nc.tensor.transpose(
                    pt[:], n_sb[b][:, 128 * fc : 128 * (fc + 1)], ident[:]
                )
                nc.scalar.copy(nT[:, fc, T * b : T * (b + 1)], pt[:])

            def emit_phase_a(b):
                """All-DVE: w-STT chain, then g in two half-D STT chains
                (first 4 nT chunks transpose early), then f maxes."""
                wvec = awp.tile([T, NP], F32, tag=f"wvec{b}",
                                name=f"wvec{b}")
                for c in range(NP):
                    nc.vector.scalar_tensor_tensor(
                        out=scr[:], in0=feats[b][c][:], scalar=1.0,
                        in1=u_sb[:], op0=mult, op1=mult,
                        accum_out=wvec[:, c : c + 1],
                    )
                for h in range(2):
                    hs = slice(512 * h, 512 * (h + 1))
                    g_ap = n_sb[b][:, 512 * h : 512 * (h + 1)]
                    nc.vector.tensor_scalar_mul(
                        g_ap, feats[b][0][:, hs], wvec[:, 0:1]
                    )
                    for c in range(1, NP):
                        nc.vector.scalar_tensor_tensor(
                            out=g_ap, in0=feats[b][c][:, hs],
                            scalar=wvec[:, c : c + 1], in1=g_ap,
                            op0=mult, op1=add,
                        )
                    for fc in range(4 * h, 4 * (h + 1)):
                        emit_transp(b, fc)
                f_ap = n_sb[b][:, D:]
                nc.vector.tensor_max(f_ap, feats[b][0][:], feats[b][1][:])
                for c in range(2, NP):
                    nc.vector.tensor_max(f_ap, f_ap, feats[b][c][:])
                for fc in range(8, NFC):
                    emit_transp(b, fc)
                wvecs.append(wvec)

            for b in range(NB):
                emit_phase_a(b)

            # ---------------- Matmul passes ----------------
            def emit_pass1(whs, tp2):
                """tp2[:, 4qc+e4, :] = (Wt|Wp quarter).T-proj, e-major."""
                for qc in range(4):
                    for e4 in range(4):
                        p1 = bigp.tile([128, NB * T], F32, tag="big",
                                       name=f"p1_{qc}_{e4}")
                        for i, fc in enumerate(FC_ORDER):
                            nc.tensor.matmul(
                                p1[:],
                                whs[qc][:, fc, 128 * e4 : 128 * (e4 + 1)],
                                nT[:, fc, :],
                                start=(i == 0),
                                stop=(i == NFC - 1),
                            )
                        nc.scalar.copy(tp2[:, 4 * qc + e4, :], p1[:])

            def emit_pass2(whs, b, gb16):
                """gb16[b] = n_b @ Wg.T  (t-major)."""
                for qc in range(4):
                    psg = bigp.tile([128, 512], F32, tag="big",
                                    name=f"psg{b}_{qc}")
                    for i, fc in enumerate(FC_ORDER):
                        nc.tensor.matmul(
                            psg[:],
                            nT[:, fc, T * b : T * (b + 1)],
                            whs[qc][:, fc, :],
                            start=(i == 0),
                            stop=(i == NFC - 1),
                        )
                    nc.scalar.copy(
                        gb16[b][:, 512 * qc : 512 * (qc + 1)], psg[:]
                    )

            def emit_pass3(tp2, gb16, mask_sb, first):
                for b in range(NB):
                    psb = psbp.tile([T, T], F32, tag="psb", name="psb")
                    for ec in range(8):
                        nc.tensor.matmul(
                            psb[:],
                            tp2[:, 8 + ec, T * b : T * (b + 1)],
                            tp2[:, ec, T * b : T * (b + 1)],
                            start=(ec == 0),
                            stop=(ec == 7),
                        )
                    sbm = sbp.tile([T, T], F16, tag="sbm", name="sbm")
                    nc.vector.scalar_tensor_tensor(
                        out=sbm[:], in0=psb[:], scalar=1.0, in1=mask_sb[:],
                        op0=mult, op1=mult,
                    )
                    for h4 in range(4):
                        po = bigp.tile([T, 512], F32, tag="big",
                                       name=f"po{b}_{h4}")
                        nc.tensor.matmul(
                            po[:],
                            sbm[:],
                            gb16[b][:, 512 * h4 : 512 * (h4 + 1)],
                            start=True,
                            stop=True,
                        )
                        sl = slice(512 * h4, 512 * (h4 + 1))
                        if first:
                            nc.vector.tensor_add(
                                out32[b][:, sl], n_sb[b][:, sl], po[:]
                            )
                        else:
                            nc.vector.tensor_add(
                                out32[b][:, sl], out32[b][:, sl], po[:]
                            )

            tp2_b = drp.tile([128, 16, NB * T], F16, tag="tp2b", name="tp2b")
            gb16_b = [drp.tile([T, F], F16, tag=f"gb{b}b", name=f"gb{b}b")
                      for b in range(NB)]
            tp2_a = drp.tile([128, 16, NB * T], F16, tag="tp2a", name="tp2a")
            gb16_a = [drp.tile([T, F], F16, tag=f"gb{b}a", name=f"gb{b}a")
                      for b in range(NB)]
            out32 = [outp.tile([T, F], F32, tag=f"out{b}", name=f"out{b}")
                     for b in range(NB)]

            emit_pass2(wg_b_h, 0, gb16_b)
            emit_pass2(wg_b_h, 1, gb16_b)
            emit_pass2(wg_a_h, 0, gb16_a)
            emit_pass2(wg_a_h, 1, gb16_a)
            emit_pass1(wtp_b_h, tp2_b)
            emit_pass3(tp2_b, gb16_b, mb_sb, first=True)
            emit_pass1(wtp_a_h, tp2_a)
            emit_pass3(tp2_a, gb16_a, ma_sb, first=False)

            for b in range(NB):
                nc.sync.dma_start(out=outd[b], in_=out32[b][:])

    nc.compile()
    return nc


def _host_prep(features, Wq, Wtb, Wpb, Wgb, Wta, Wpa, Wga):
    f32 = np.float32
    f16 = np.float16
    # [B, T, NP, D] -> [B, NP, T, D] n-major for chunked phase A
    feat = np.ascontiguousarray(
        np.asarray(features, f32).transpose(0, 2, 1, 3).astype(f16)
    )
    u = np.asarray(Wq, f32).sum(axis=0)[None, :]

    def wt(w):  # [e, f] -> [f, e] fp16 contiguous
        return np.ascontiguousarray(np.asarray(w, f32).T.astype(f16))

    def pack(w):  # [F, E] -> [E//512, 128, NFC, 512] SBUF-layout quarters
        q = w.reshape(NFC, 128, -1)  # [c, p, E]
        ne = q.shape[2] // 512
        # quarter qc: rows (c p), cols 512*qc:512*(qc+1) -> [p, c, 512]
        out = np.empty((ne, 128, NFC, 512), f16)
        for qc in range(ne):
            out[qc] = q[:, :, 512 * qc : 512 * (qc + 1)].transpose(1, 0, 2)
        return np.ascontiguousarray(out)

    wtp_b = pack(np.concatenate([wt(Wtb), wt(Wpb)], axis=1))
    wtp_a = pack(np.concatenate([wt(Wta), wt(Wpa)], axis=1))
    wg_b = pack(wt(Wgb))
    wg_a = pack(wt(Wga))

    idx = np.arange(T)
    # masks indexed [j, i] (psb holds sb transposed)
    maskb = (SCALE * (idx[None, :] > idx[:, None])).astype(f32)
    maska = (SCALE * (idx[None, :] < idx[:, None])).astype(f32)
    ident = np.eye(128, dtype=f16)

    shared = {
        "u": u.astype(f16),
        "maskb": maskb,
        "maska": maska,
        "ident": ident,
        "wtp_b": wtp_b,
        "wg_b": wg_b,
        "wtp_a": wtp_a,
        "wg_a": wg_a,
    }
    feat16 = feat.reshape(NCORES, NB, NP, T, D)
    return shared, feat16


def kernel(**inputs) -> np.ndarray:
    if "nc" not in _CACHE:
        _CACHE["nc"] = _build()
    nc = _CACHE["nc"]

    shared, feat16 = _host_prep(**inputs)
    in_maps = [dict(shared, feat=feat16[c]) for c in range(NCORES)]
    res = run_bass_kernel_spmd(
        nc, in_maps, core_ids=list(range(NCORES)), trace=_PROFILE["trace"]
    )
    _PROFILE["result"] = res
    out = np.stack([res.results[c]["out"] for c in range(NCORES)], axis=0)
    return out.reshape(B, T, F).astype(np.float32)
